# revision 21
# baseline (speedup 1.0000x reference)
"""DGCNN layer (dynamic kNN graph + edge MLP) for 8 Trainium2 cores.

Screen: packed (quantized score, in-window index) kNN screen — m1 (fp16,
K=70) computes CSCALE*(score+80) + M with M = 1.5*2^32 forcing fp32 RNE
quantization to multiples of 512; m2 (-M cancel via PSUM RMW) and m3
(+iota) complete the pack; one DVE max8 pass per 512-wide window then
yields top-8 packed (score, index) values. The three matmuls of each
window chain are software-pipelined across PSUM banks (m1(w), m2(w-1),
m3(w-2), ...) so consecutive PE instructions never RMW the same bank.

Edge MLP: the per-edge neighbor fetch is a hardware-DGE dma_gather of
padded x rows (256B each) from HBM with transpose=True (max 512 indices
per call — larger wedges the exec unit), landing [128, edges] fp16
columns directly; q_j (+b1, folded via the padded row's constant-1
column) is computed per-chunk on the PE. Per-block index transposes run
on the PE (is_transpose matmul against an identity), so each block's
MLP starts right after its screen — no half-barrier, no serialized
gather tail on the single SWDGE queue.
"""

import os
import sys

import numpy as np

N, D, C, K = 16384, 64, 128, 16
NCORES = 8
RPC = N // NCORES          # rows per core
BLK = 128                  # rows per screen block
WIN = 512                  # screen window (one PSUM bank of fp32)
CHUNK = 512                # edges per MLP chunk
NMAG = 8                   # magic rows in m1
KA = D + 2 + NMAG          # m1 contraction rows (x + sq hi/lo + magic)

CSCALE = 98304.0           # score scale; quantum = QW/CSCALE = 1/96
SHIFT = 80.0               # recenters scores so |y| stays under 2^24
MAGW = 32768.0             # magic piece = MAGW * MAGR = 1.5*2^30; 8 pieces = 1.5*2^33
MAGR = 49152.0
QW = 1024                  # coarse window: one max8 spans two PSUM banks
MAGF = float(1.5 * 2.0**23)  # fp32 integer-rounding constant for DVE
NEG = -3.0e38
MARK = float(1 << 20)
PE_ROUTE = 7               # of every 16 windows, this many fix up on the PE

_REPO = "/opt/trn_rl_repo"


def _ensure_path():
    if _REPO not in sys.path:
        sys.path.insert(0, _REPO)


def build_program(n=N, d=D, c=C, k=K, rpc=RPC):
    _ensure_path()
    import concourse.mybir as mybir
    from concourse import tile
    from concourse.bacc import Bacc

    f32 = mybir.dt.float32
    f16 = mybir.dt.float16
    i16 = mybir.dt.int16

    nblk = rpc // BLK                    # 16
    nwin = n // WIN                      # 32
    nqwin = n // QW                      # 16
    ncoarse = nqwin * 8                  # 128 coarse slots per row
    rows_per_chunk = CHUNK // k          # 32
    chunks_per_blk = BLK // rows_per_chunk  # 4

    nc = Bacc()

    xaug_d = nc.declare_dram_parameter("xaug", [KA, n], f16, isOutput=False)
    wloc_d = nc.declare_dram_parameter("wloc", [KA, rpc], f16, isOutput=False)
    unmw_d = nc.declare_dram_parameter("unmw", [NMAG, 128], f16, isOutput=False)
    unmr_d = nc.declare_dram_parameter("unmr", [NMAG, WIN], f16, isOutput=False)
    iow_d = nc.declare_dram_parameter("iow", [1, 128], f16, isOutput=False)
    ior_d = nc.declare_dram_parameter("ior", [1, QW], f16, isOutput=False)
    eye_d = nc.declare_dram_parameter("eye", [128, 128], f32, isOutput=False)
    iot_d = nc.declare_dram_parameter("iot", [128, QW], f32, isOutput=False)
    xpad_d = nc.declare_dram_parameter("xpad", [n, 128], f16, isOutput=False)
    w1bp_d = nc.declare_dram_parameter("w1bp", [128, c], f16, isOutput=False)
    w1d_d = nc.declare_dram_parameter("w1d", [d, c], f16, isOutput=False)
    w2_d = nc.declare_dram_parameter("w2", [c, c], f16, isOutput=False)
    b2s_d = nc.declare_dram_parameter("b2s", [c, 1], f32, isOutput=False)
    wbase_d = nc.declare_dram_parameter("wbase", [128, ncoarse], f32, isOutput=False)
    oneh_d = nc.declare_dram_parameter("oneh", [128, CHUNK], f16, isOutput=False)
    out_d = nc.declare_dram_parameter("outT", [c, rpc], f32, isOutput=True)

    with tile.TileContext(nc) as tc:
        with (
            tc.tile_pool(name="const", bufs=1) as cpool,
            tc.tile_pool(name="screen", bufs=2) as spool,
            tc.tile_pool(name="small", bufs=2) as mpool,
            tc.tile_pool(name="mlp", bufs=3) as dpool,
            tc.tile_pool(name="xsel", bufs=10) as qpool,
            tc.tile_pool(name="evac", bufs=3) as epool,
            tc.tile_pool(name="psA", bufs=3, space="PSUM") as ppA,
            tc.tile_pool(name="psM", bufs=2, space="PSUM") as ppM,
        ):
            # ---- persistent tiles ----
            xaug = cpool.tile([KA, n], f16, tag="xaug")
            wloc = cpool.tile([KA, rpc], f16, tag="wloc")
            unmw = cpool.tile([NMAG, 128], f16, tag="unmw")
            unmr = cpool.tile([NMAG, WIN], f16, tag="unmr")
            iow = cpool.tile([1, 128], f16, tag="iow")
            ior = cpool.tile([1, QW], f16, tag="ior")
            eye = cpool.tile([128, 128], f32, tag="eye")
            iotarep = cpool.tile([128, QW], f32, tag="iotarep")
            w1bp = cpool.tile([128, c], f16, tag="w1bp")
            w1d = cpool.tile([d, c], f16, tag="w1d")
            w2 = cpool.tile([c, c], f16, tag="w2")
            b2s = cpool.tile([c, 1], f32, tag="b2s")
            wbase = cpool.tile([128, ncoarse], f32, tag="wbase")
            pTT = cpool.tile([128, nblk * c], f16, tag="pTT")
            onehot4 = cpool.tile([128, CHUNK], f16, tag="onehot4")
            outT = cpool.tile([c, rpc], f32, tag="outT")
            jrep = [
                cpool.tile([128, BLK], i16, tag=f"jrep{b}", name=f"jrep{b}")
                for b in range(nblk)
            ]

            # small/critical params first (wloc gates phase A and the
            # screen); the 2.3MB xaug streams in behind, split so window 0
            # can start early
            for t, dd in [(wloc, wloc_d), (w1d, w1d_d), (unmw, unmw_d),
                          (unmr, unmr_d), (iow, iow_d), (ior, ior_d),
                          (eye, eye_d), (iotarep, iot_d),
                          (w1bp, w1bp_d), (w2, w2_d),
                          (b2s, b2s_d), (wbase, wbase_d),
                          (onehot4, oneh_d)]:
                nc.sync.dma_start(t[:, :], dd[:, :])
            for qh in range(8):
                nc.sync.dma_start(
                    xaug[:, qh * (n // 8):(qh + 1) * (n // 8)],
                    xaug_d[:, qh * (n // 8):(qh + 1) * (n // 8)],
                )

            nidx_reg = nc.gpsimd.to_reg(CHUNK)

            # ---- phase A: pTT[t, c] = x_t.(W1a-W1b), transposed + fp16 so
            # the per-chunk one-hot matmul can use it as stationary ----
            for b in range(nblk):
                pp = ppM.tile([BLK, c], f32, tag="mlpps")
                nc.tensor.matmul(pp[:, :], wloc[0:d, b * BLK:(b + 1) * BLK],
                                 w1d[:, :])
                nc.scalar.activation(
                    pTT[:, b * c:(b + 1) * c], pp[:, :],
                    mybir.ActivationFunctionType.Copy,
                )

            def screen_block(b):
                """Packed screen windows for rows [b*BLK, (b+1)*BLK) -> cP.

                Fixups (-M cancel, +iota) are split per window between the
                PE route (m2/m3 matmuls, max8 reads PSUM) and the ACT+DVE
                route (ACT evacuates PSUM with bias=-M — exact by Sterbenz,
                M and M+512q share a binade — then DVE adds iota in SBUF)
                to balance the engine fleet; PE_ROUTE of every 16 windows
                go to the PE."""
                cP = spool.tile([128, ncoarse], f32, tag="cP")
                wsl = wloc[:, b * BLK:(b + 1) * BLK]
                for q in range(nqwin):
                    ps = ppA.tile([128, QW], f32, tag="scr")
                    if (q * PE_ROUTE) % 16 < PE_ROUTE:  # evenly interleaved split
                        for h in range(2):
                            hs = ps[:, h * WIN:(h + 1) * WIN]
                            xw = xaug[:, (2 * q + h) * WIN:(2 * q + h + 1) * WIN]
                            nc.tensor.matmul(hs, wsl, xw,
                                             start=True, stop=False)
                            nc.tensor.matmul(hs, unmw[:, :], unmr[:, :],
                                             start=False, stop=False)
                            nc.tensor.matmul(hs, iow[:, :],
                                             ior[:, h * WIN:(h + 1) * WIN],
                                             start=False, stop=True)
                        nc.vector.max(cP[:, 8 * q:8 * q + 8], ps[:, :])
                    else:
                        for h in range(2):
                            nc.tensor.matmul(
                                ps[:, h * WIN:(h + 1) * WIN], wsl,
                                xaug[:, (2 * q + h) * WIN:(2 * q + h + 1) * WIN],
                                start=True, stop=True)
                        sb = epool.tile([128, QW], f32, tag="ev")
                        nc.scalar.activation(
                            sb[:, :], ps[:, :],
                            mybir.ActivationFunctionType.Copy,
                            bias=-float(NMAG) * MAGW * MAGR,
                        )
                        nc.vector.tensor_tensor(
                            out=sb[:, :], in0=sb[:, :], in1=iotarep[:, :],
                            op=mybir.AluOpType.add,
                        )
                        nc.vector.max(cP[:, 8 * q:8 * q + 8], sb[:, :])
                return cP

            def extract_block(b, cP):
                """Top-16 (index-packed) selection from cP -> j16f.

                Emitted one block behind the screen so this serial
                ACT<->DVE chain never stalls either in-order queue.

                extraction: gj = wbase + (P mod 512), exact fp32 int ops;
                affine pieces run on ACT to keep DVE off the critical path"""
                t1 = mpool.tile([128, ncoarse], f32, tag="t1")
                t2 = mpool.tile([128, ncoarse], f32, tag="t2")
                dfr = mpool.tile([128, ncoarse], f32, tag="dfr")
                gj = mpool.tile([128, ncoarse], f32, tag="gj")
                nc.scalar.activation(t1[:, :], cP[:, :],
                                     mybir.ActivationFunctionType.Copy,
                                     scale=1.0 / QW)
                nc.scalar.activation(t2[:, :], t1[:, :],
                                     mybir.ActivationFunctionType.Copy,
                                     bias=MAGF)
                nc.scalar.activation(t2[:, :], t2[:, :],
                                     mybir.ActivationFunctionType.Copy,
                                     bias=-MAGF)
                nc.vector.tensor_tensor(out=dfr[:, :], in0=t1[:, :], in1=t2[:, :],
                                        op=mybir.AluOpType.subtract)
                # gj = 512*d + 512*(d<0) + wbase
                nc.vector.tensor_scalar(t2[:, :], dfr[:, :], 0.0, float(QW),
                                        op0=mybir.AluOpType.is_lt,
                                        op1=mybir.AluOpType.mult)
                nc.scalar.activation(t1[:, :], dfr[:, :],
                                     mybir.ActivationFunctionType.Copy,
                                     scale=float(QW))
                nc.vector.tensor_tensor(out=gj[:, :], in0=t1[:, :], in1=t2[:, :],
                                        op=mybir.AluOpType.add)
                nc.vector.tensor_tensor(out=gj[:, :], in0=gj[:, :], in1=wbase[:, :],
                                        op=mybir.AluOpType.add)

                # mark top-16 coarse slots in-place (by packed value)
                m8a = mpool.tile([128, 8], f32, tag="m8a")
                m8b = mpool.tile([128, 8], f32, tag="m8b")
                zap = mpool.tile([128, ncoarse], f32, tag="zap")
                nc.vector.max(m8a[:, :], cP[:, :])
                nc.vector.match_replace(zap[:, :], m8a[:, :], cP[:, :], NEG)
                nc.vector.max(m8b[:, :], zap[:, :])
                nc.vector.match_replace(zap[:, :], m8b[:, :], zap[:, :], NEG)

                # compact: packed = 2^20 * is_marked + gj, top-16 of packed
                mask = mpool.tile([128, ncoarse], f32, tag="mask")
                nc.vector.tensor_scalar(
                    mask[:, :], zap[:, :], -1.0e38, MARK,
                    op0=mybir.AluOpType.is_le, op1=mybir.AluOpType.mult,
                )
                nc.vector.tensor_tensor(out=mask[:, :], in0=mask[:, :], in1=gj[:, :],
                                        op=mybir.AluOpType.add)
                p8a = mpool.tile([128, 8], f32, tag="p8a")
                p8b = mpool.tile([128, 8], f32, tag="p8b")
                nc.vector.max(p8a[:, :], mask[:, :])
                nc.vector.match_replace(mask[:, :], p8a[:, :], mask[:, :], NEG)
                nc.vector.max(p8b[:, :], mask[:, :])

                j16f = mpool.tile([128, 2 * 8], f32, tag="j16f")
                nc.scalar.activation(j16f[:, 0:8], p8a[:, :],
                                     mybir.ActivationFunctionType.Copy,
                                     bias=-MARK)
                nc.scalar.activation(j16f[:, 8:16], p8b[:, :],
                                     mybir.ActivationFunctionType.Copy,
                                     bias=-MARK)
                return j16f

            def transpose_block(b, j16f):
                """PE-transpose j16f [128,16] -> [16,128], cast to i16,
                replicate into jrep[b] (8 copies, both HWDGE queues)."""
                psT = ppM.tile([k, 128], f32, tag="mlpps")
                nc.tensor.matmul(psT[:, :], j16f[:, :], eye[:, :],
                                 is_transpose=True)
                jt = mpool.tile([k, 128], i16, tag="jt16")
                nc.vector.tensor_copy(jt[:, :], psT[:, :])
                for g in range(8):
                    eng = nc.sync if g % 2 == 0 else nc.scalar
                    eng.dma_start(jrep[b][16 * g:16 * g + k, :], jt[:, :])

            def mlp_gather(b):
                """Issue the 4 neighbor gathers for one block (lag-1, so
                the single SWDGE queue has a full block of lead time).

                dma_gather(transpose=True) is limited to 512 indices per
                call (hardware-bisected: 1024+ wedges the exec unit), so
                gather per 512-edge chunk."""
                xsels = []
                for sub in range(chunks_per_blk):
                    xsel = qpool.tile([128, CHUNK], f16, tag="xsel")
                    nc.gpsimd.dma_gather(
                        xsel[:, :].rearrange("p (o i) -> p o i", o=1),
                        xpad_d[:, :],
                        jrep[b][:, sub * rows_per_chunk:(sub + 1) * rows_per_chunk],
                        CHUNK, nidx_reg, 128,
                        transpose=True,
                    )
                    xsels.append(xsel)
                return xsels

            def mlp_block(b, xsels):
                """Edge MLP for one block (lag-2 behind its screen)."""
                for sub in range(chunks_per_blk):
                    r0 = b * BLK + sub * rows_per_chunk
                    xsel = xsels[sub]
                    qps = ppM.tile([c, CHUNK], f32, tag="mlpps")
                    nc.tensor.matmul(qps[:, :], w1bp[:, :], xsel[:, :],
                                     start=True, stop=False)
                    # += p_i via static one-hot: row sub*32+t selects cols
                    # [16t,16t+16) -> adds pTT[r0+t, :] to those edges
                    nc.tensor.matmul(
                        qps[:, :],
                        pTT[sub * 32:(sub + 1) * 32, b * c:(b + 1) * c],
                        onehot4[sub * 32:(sub + 1) * 32, :],
                        start=False, stop=True,
                        tile_position=(sub * 32, 0),
                    )
                    h1 = dpool.tile([128, CHUNK], f16, tag="h1")
                    nc.scalar.activation(
                        h1[:, :], qps[:, :], mybir.ActivationFunctionType.Relu,
                    )
                    ps2 = ppM.tile([128, CHUNK], f32, tag="mlpps")
                    nc.tensor.matmul(ps2[:, :], w2[:, :], h1[:, :])
                    h2 = dpool.tile([128, CHUNK], f32, tag="h2")
                    nc.scalar.activation(
                        h2[:, :], ps2[:, :], mybir.ActivationFunctionType.Relu,
                        bias=b2s[:, :], scale=1.0 / k,
                    )
                    nc.vector.tensor_reduce(
                        out=outT[:, r0:r0 + rows_per_chunk],
                        in_=h2[:, :].rearrange("p (r k) -> p r k", k=k),
                        op=mybir.AluOpType.add,
                        axis=mybir.AxisListType.X,
                    )

            pend = {}
            for b in range(nblk + 2):
                if b < nblk:
                    cP = screen_block(b)
                    pend[b] = [cP, None]
                if b - 1 >= 0 and b - 1 < nblk:
                    j16f = extract_block(b - 1, pend[b - 1][0])
                    transpose_block(b - 1, j16f)
                    pend[b - 1][1] = mlp_gather(b - 1)
                if b - 2 >= 0:
                    mlp_block(b - 2, pend[b - 2][1])
                    del pend[b - 2]

            nc.sync.dma_start(out_d[:, :], outT[:, :])

    nc.finalize()
    return nc


def host_prep(x, W1, b1, W2, b2, n=N, d=D, c=C, k=K, rpc=RPC, ncores=NCORES):
    x = np.ascontiguousarray(np.asarray(x, dtype=np.float32))
    W1 = np.asarray(W1, dtype=np.float32)
    b1 = np.asarray(b1, dtype=np.float32)
    W2 = np.asarray(W2, dtype=np.float32)
    b2 = np.asarray(b2, dtype=np.float32)

    sq = np.sum(x * x, axis=1, dtype=np.float32)
    nqwin = n // QW
    ncoarse = nqwin * 8
    a = np.float32(np.sqrt(2.0 * CSCALE))

    # moving operand: rows 0:d = a*x^T; d,d+1 = (sq-SHIFT) hi/lo; d+2.. = magic
    s = (sq - np.float32(SHIFT)).astype(np.float32)
    sh = s.astype(np.float16)
    sl = (s.astype(np.float64) - sh.astype(np.float64)).astype(np.float16)
    xaug = np.zeros((KA, n), dtype=np.float16)
    xaug[:d] = (x.T * a).astype(np.float16)
    # sq rows are applied with weight -CSCALE/2 (fp16 can't hold -98304),
    # so carry 2*s here; x2 is exact in fp16
    xaug[d] = 2.0 * sh
    xaug[d + 1] = 2.0 * sl
    xaug[d + 2:d + 2 + NMAG] = np.float16(MAGR)

    unmw = np.full((NMAG, 128), -MAGW, dtype=np.float16)
    unmr = np.full((NMAG, WIN), MAGR, dtype=np.float16)
    iow = np.ones((1, 128), dtype=np.float16)
    ior = np.arange(QW, dtype=np.float16).reshape(1, QW)
    eye = np.eye(128, dtype=np.float32)
    iot = np.arange(QW, dtype=np.float32)[None, :].repeat(128, axis=0)
    iot = np.ascontiguousarray(iot)

    # padded x rows for the HBM neighbor gather; col d holds the constant 1
    # that folds b1 into the per-chunk q matmul
    xpad = np.zeros((n, 128), dtype=np.float16)
    xpad[:, :d] = x.astype(np.float16)
    xpad[:, d] = np.float16(1.0)
    w1bp = np.zeros((128, c), dtype=np.float16)
    w1bp[:d] = W1[d:].astype(np.float16)
    w1bp[d] = b1.astype(np.float16)

    w1d = ((W1[:d] - W1[d:]) / a).astype(np.float16)
    oneh = np.zeros((128, CHUNK), dtype=np.float16)
    for p_ in range(128):
        t_ = p_ % 32
        oneh[p_, 16 * t_:16 * t_ + 16] = 1.0
    w2 = W2.astype(np.float16)
    b2s = (b2 / k).reshape(c, 1).astype(np.float32)
    wbase = np.repeat(
        (np.arange(nqwin, dtype=np.float32) * QW), 8
    )[None, :].repeat(128, axis=0).astype(np.float32)
    wbase = np.ascontiguousarray(wbase[:, :ncoarse])

    in_maps = []
    for cid in range(ncores):
        rows = x[cid * rpc:(cid + 1) * rpc]
        wloc = np.zeros((KA, rpc), dtype=np.float16)
        wloc[:d] = (rows.T * a).astype(np.float16)
        wloc[d] = np.float16(-CSCALE / 2)
        wloc[d + 1] = np.float16(-CSCALE / 2)
        wloc[d + 2:d + 2 + NMAG] = np.float16(MAGW)
        in_maps.append(
            dict(
                xaug=xaug, wloc=np.ascontiguousarray(wloc), unmw=unmw,
                unmr=unmr, iow=iow, ior=ior, eye=eye,
                iot=iot,
                xpad=xpad, w1bp=w1bp, w1d=w1d, w2=w2, b2s=b2s,
                wbase=wbase, oneh=oneh,
            )
        )
    return in_maps


_NC_CACHE = {}


def kernel(x, W1, b1, W2, b2):
    _ensure_path()
    from concourse.bass_utils import run_bass_kernel_spmd

    key = "full"
    if key not in _NC_CACHE:
        _NC_CACHE[key] = build_program()
    nc = _NC_CACHE[key]

    in_maps = host_prep(x, W1, b1, W2, b2)
    res = run_bass_kernel_spmd(
        nc, in_maps, core_ids=list(range(NCORES)),
        trace=bool(int(os.environ.get("DGCNN_TRACE", "0"))),
    )
    out = np.empty((N, C), dtype=np.float32)
    for cid in range(NCORES):
        out[cid * RPC:(cid + 1) * RPC] = res.results[cid]["outT"].T
    if getattr(res, "exec_time_ns", None):
        kernel.last_exec_time_ns = res.exec_time_ns
    return out


kernel.last_exec_time_ns = None


# revision 23
# speedup vs baseline: 1.3237x; 1.3237x over previous
"""DGCNN layer (dynamic kNN graph + edge MLP) for 8 Trainium2 cores.

Screen: packed (quantized score, in-window index) kNN screen — m1 (fp16,
K=70) computes CSCALE*(score+80) + M with M = 1.5*2^32 forcing fp32 RNE
quantization to multiples of 512; m2 (-M cancel via PSUM RMW) and m3
(+iota) complete the pack; one DVE max8 pass per 512-wide window then
yields top-8 packed (score, index) values. The three matmuls of each
window chain are software-pipelined across PSUM banks (m1(w), m2(w-1),
m3(w-2), ...) so consecutive PE instructions never RMW the same bank.

Edge MLP: the per-edge neighbor fetch is a hardware-DGE dma_gather of
padded x rows (256B each) from HBM with transpose=True (max 512 indices
per call — larger wedges the exec unit), landing [128, edges] fp16
columns directly; q_j (+b1, folded via the padded row's constant-1
column) is computed per-chunk on the PE. Per-block index transposes run
on the PE (is_transpose matmul against an identity), so each block's
MLP starts right after its screen — no half-barrier, no serialized
gather tail on the single SWDGE queue.
"""

import os
import sys

import numpy as np

N, D, C, K = 16384, 64, 128, 16
NCORES = 8
RPC = N // NCORES          # rows per core
BLK = 128                  # rows per screen block
WIN = 512                  # screen window (one PSUM bank of fp32)
CHUNK = 512                # edges per MLP chunk
KA = D + 2 + 4             # m1 contraction rows (x + sq hi/lo + magic)

CSCALE = 61440.0           # score scale; quantum = 512/CSCALE
SHIFT = 80.0               # recenters scores so |y| stays under 2^23
MAGW = 32768.0             # magic piece = MAGW * MAGR = 3*2^29; 4 pieces = 1.5*2^32
MAGR = 49152.0
MAGF = float(1.5 * 2.0**23)  # fp32 integer-rounding constant for DVE
NEG = -3.0e38
MARK = float(1 << 20)
PE_ROUTE = 7               # of every 16 windows, this many fix up on the PE

_REPO = "/opt/trn_rl_repo"


def _ensure_path():
    if _REPO not in sys.path:
        sys.path.insert(0, _REPO)


def build_program(n=N, d=D, c=C, k=K, rpc=RPC):
    _ensure_path()
    import concourse.mybir as mybir
    from concourse import tile
    from concourse.bacc import Bacc

    f32 = mybir.dt.float32
    f16 = mybir.dt.float16
    i16 = mybir.dt.int16
    u16 = mybir.dt.uint16

    nblk = rpc // BLK                    # 16
    nwin = n // WIN                      # 32
    ncoarse = nwin * 8                   # 256 coarse slots per row
    rows_per_chunk = CHUNK // k          # 32
    chunks_per_blk = BLK // rows_per_chunk  # 4

    nc = Bacc()

    xaug_d = nc.declare_dram_parameter("xaug", [KA, n], f16, isOutput=False)
    wloc_d = nc.declare_dram_parameter("wloc", [KA, rpc], f16, isOutput=False)
    unmw_d = nc.declare_dram_parameter("unmw", [4, 128], f16, isOutput=False)
    unmr_d = nc.declare_dram_parameter("unmr", [4, WIN], f16, isOutput=False)
    iow_d = nc.declare_dram_parameter("iow", [1, 128], f16, isOutput=False)
    ior_d = nc.declare_dram_parameter("ior", [1, WIN], f16, isOutput=False)
    eye_d = nc.declare_dram_parameter("eye", [128, 128], f32, isOutput=False)
    iot_d = nc.declare_dram_parameter("iot", [128, WIN], f32, isOutput=False)
    xpad_d = nc.declare_dram_parameter("xpad", [n, 128], f16, isOutput=False)
    w1bp_d = nc.declare_dram_parameter("w1bp", [128, c], f16, isOutput=False)
    w1d_d = nc.declare_dram_parameter("w1d", [d, c], f16, isOutput=False)
    w2_d = nc.declare_dram_parameter("w2", [c, c], f16, isOutput=False)
    b2s_d = nc.declare_dram_parameter("b2s", [c, 1], f32, isOutput=False)
    oneh_d = nc.declare_dram_parameter("oneh", [128, CHUNK], f16, isOutput=False)
    out_d = nc.declare_dram_parameter("outT", [c, rpc], f32, isOutput=True)

    with tile.TileContext(nc) as tc:
        with (
            tc.tile_pool(name="const", bufs=1) as cpool,
            tc.tile_pool(name="screen", bufs=2) as spool,
            tc.tile_pool(name="small", bufs=2) as mpool,
            tc.tile_pool(name="mlp", bufs=3) as dpool,
            tc.tile_pool(name="xsel", bufs=10) as qpool,
            tc.tile_pool(name="evac", bufs=3) as epool,
            tc.tile_pool(name="psA", bufs=4, space="PSUM") as ppA,
            tc.tile_pool(name="psQ", bufs=2, space="PSUM") as ppQ,
            tc.tile_pool(name="psB", bufs=2, space="PSUM") as ppB,
        ):
            # ---- persistent tiles ----
            xaug = cpool.tile([KA, n], f16, tag="xaug")
            wloc = cpool.tile([KA, rpc], f16, tag="wloc")
            unmw = cpool.tile([4, 128], f16, tag="unmw")
            unmr = cpool.tile([4, WIN], f16, tag="unmr")
            iow = cpool.tile([1, 128], f16, tag="iow")
            ior = cpool.tile([1, WIN], f16, tag="ior")
            eye = cpool.tile([128, 128], f32, tag="eye")
            iotarep = cpool.tile([128, WIN], f32, tag="iotarep")
            w1bp = cpool.tile([128, c], f16, tag="w1bp")
            w1d = cpool.tile([d, c], f16, tag="w1d")
            w2 = cpool.tile([c, c], f16, tag="w2")
            b2s = cpool.tile([c, 1], f32, tag="b2s")
            pTT = cpool.tile([128, nblk * c], f16, tag="pTT")
            onehot4 = cpool.tile([128, CHUNK], f16, tag="onehot4")
            outT = cpool.tile([c, rpc], f32, tag="outT")
            jrep = [
                cpool.tile([128, BLK], i16, tag=f"jrep{b}", name=f"jrep{b}")
                for b in range(nblk)
            ]

            # small/critical params first (wloc gates phase A and the
            # screen); the 2.3MB xaug streams in behind, split so window 0
            # can start early
            for t, dd in [(wloc, wloc_d), (w1d, w1d_d), (unmw, unmw_d),
                          (unmr, unmr_d), (iow, iow_d), (ior, ior_d),
                          (eye, eye_d), (iotarep, iot_d),
                          (w1bp, w1bp_d), (w2, w2_d),
                          (b2s, b2s_d),
                          (onehot4, oneh_d)]:
                nc.sync.dma_start(t[:, :], dd[:, :])
            for qh in range(8):
                nc.sync.dma_start(
                    xaug[:, qh * (n // 8):(qh + 1) * (n // 8)],
                    xaug_d[:, qh * (n // 8):(qh + 1) * (n // 8)],
                )

            nidx_reg = nc.gpsimd.to_reg(CHUNK)

            # ---- phase A: pTT[t, c] = x_t.(W1a-W1b), transposed + fp16 so
            # the per-chunk one-hot matmul can use it as stationary ----
            for b in range(nblk):
                pp = ppB.tile([BLK, c], f32, tag="mm2")
                nc.tensor.matmul(pp[:, :], wloc[0:d, b * BLK:(b + 1) * BLK],
                                 w1d[:, :])
                nc.scalar.activation(
                    pTT[:, b * c:(b + 1) * c], pp[:, :],
                    mybir.ActivationFunctionType.Copy,
                )

            def screen_block(b):
                """Packed screen windows for rows [b*BLK, (b+1)*BLK) -> cP.

                Fixups (-M cancel, +iota) are split per window between the
                PE route (m2/m3 matmuls, max8 reads PSUM) and the ACT+DVE
                route (ACT evacuates PSUM with bias=-M — exact by Sterbenz,
                M and M+512q share a binade — then DVE adds iota in SBUF)
                to balance the engine fleet; PE_ROUTE of every 16 windows
                go to the PE."""
                cP = spool.tile([128, ncoarse], f32, tag="cP")
                wsl = wloc[:, b * BLK:(b + 1) * BLK]
                for w in range(nwin):
                    ps = ppA.tile([128, WIN], f32, tag="scr")
                    xw = xaug[:, w * WIN:(w + 1) * WIN]
                    if (w * PE_ROUTE) % 16 < PE_ROUTE:  # evenly interleaved split
                        nc.tensor.matmul(ps[:, :], wsl, xw,
                                         start=True, stop=False)
                        nc.tensor.matmul(ps[:, :], unmw[:, :], unmr[:, :],
                                         start=False, stop=False)
                        nc.tensor.matmul(ps[:, :], iow[:, :], ior[:, :],
                                         start=False, stop=True)
                        nc.vector.max(cP[:, 8 * w:8 * w + 8], ps[:, :])
                    else:
                        nc.tensor.matmul(ps[:, :], wsl, xw,
                                         start=True, stop=True)
                        sb = epool.tile([128, WIN], f32, tag="ev")
                        nc.scalar.activation(
                            sb[:, :], ps[:, :],
                            mybir.ActivationFunctionType.Copy,
                            bias=-4.0 * MAGW * MAGR,
                        )
                        nc.vector.tensor_tensor(
                            out=sb[:, :], in0=sb[:, :], in1=iotarep[:, :],
                            op=mybir.AluOpType.add,
                        )
                        nc.vector.max(cP[:, 8 * w:8 * w + 8], sb[:, :])
                return cP

            def extract_block(b, cP):
                """Top-16 selection from cP via max8+max_index; the
                mod-512/window-base arithmetic runs on [128,16] arrays.

                Emitted one block behind the screen so the serial chain
                never stalls either in-order queue."""
                v16 = mpool.tile([128, 16], f32, tag="v16")
                i16 = mpool.tile([128, 16], u16, tag="i16")
                i16f = mpool.tile([128, 16], f32, tag="i16f")
                zap = mpool.tile([128, ncoarse], f32, tag="zap")
                nc.vector.max(v16[:, 0:8], cP[:, :])
                nc.vector.max_index(i16[:, 0:8], v16[:, 0:8], cP[:, :])
                nc.vector.match_replace(zap[:, :], v16[:, 0:8], cP[:, :], NEG)
                nc.vector.max(v16[:, 8:16], zap[:, :])
                nc.vector.max_index(i16[:, 8:16], v16[:, 8:16], zap[:, :])
                nc.vector.tensor_copy(i16f[:, :], i16[:, :])

                t1 = mpool.tile([128, 16], f32, tag="t1")
                t2 = mpool.tile([128, 16], f32, tag="t2")
                dfr = mpool.tile([128, 16], f32, tag="dfr")
                neg = mpool.tile([128, 16], f32, tag="neg")
                jj = mpool.tile([128, 16], f32, tag="jj")
                wb = mpool.tile([128, 16], f32, tag="wb")
                j16f = mpool.tile([128, 16], f32, tag="j16f")
                # j = P mod 512 (P may be negative; round-to-int via MAGF)
                nc.vector.tensor_scalar(t1[:, :], v16[:, :], 1.0 / WIN, None,
                                        op0=mybir.AluOpType.mult)
                nc.vector.tensor_scalar(t2[:, :], t1[:, :], MAGF, MAGF,
                                        op0=mybir.AluOpType.add,
                                        op1=mybir.AluOpType.subtract)
                nc.vector.tensor_tensor(out=dfr[:, :], in0=t1[:, :], in1=t2[:, :],
                                        op=mybir.AluOpType.subtract)
                nc.vector.tensor_scalar(neg[:, :], dfr[:, :], 0.0, float(WIN),
                                        op0=mybir.AluOpType.is_lt,
                                        op1=mybir.AluOpType.mult)
                nc.vector.tensor_scalar(jj[:, :], dfr[:, :], float(WIN), None,
                                        op0=mybir.AluOpType.mult)
                # window base = 512 * (coarse_col // 8), col//8 = rnd((col-3.5)/8)
                nc.vector.tensor_scalar(wb[:, :], i16f[:, :], 0.125, 0.4375,
                                        op0=mybir.AluOpType.mult,
                                        op1=mybir.AluOpType.subtract)
                nc.vector.tensor_scalar(wb[:, :], wb[:, :], MAGF, MAGF,
                                        op0=mybir.AluOpType.add,
                                        op1=mybir.AluOpType.subtract)
                nc.vector.tensor_scalar(wb[:, :], wb[:, :], float(WIN), None,
                                        op0=mybir.AluOpType.mult)
                nc.vector.tensor_tensor(out=jj[:, :], in0=jj[:, :], in1=neg[:, :],
                                        op=mybir.AluOpType.add)
                nc.vector.tensor_tensor(out=j16f[:, :], in0=jj[:, :], in1=wb[:, :],
                                        op=mybir.AluOpType.add)
                return j16f

            def transpose_block(b, j16f):
                """PE-transpose j16f [128,16] -> [16,128], cast to i16,
                replicate into jrep[b] (8 copies, both HWDGE queues)."""
                psT = ppQ.tile([k, 128], f32, tag="qps")
                nc.tensor.matmul(psT[:, :], j16f[:, :], eye[:, :],
                                 is_transpose=True)
                jt = mpool.tile([k, 128], i16, tag="jt16")
                nc.vector.tensor_copy(jt[:, :], psT[:, :])
                for g in range(8):
                    eng = nc.sync if g % 2 == 0 else nc.scalar
                    eng.dma_start(jrep[b][16 * g:16 * g + k, :], jt[:, :])

            def mlp_gather(b):
                """Issue the 4 neighbor gathers for one block (lag-1, so
                the single SWDGE queue has a full block of lead time).

                dma_gather(transpose=True) is limited to 512 indices per
                call (hardware-bisected: 1024+ wedges the exec unit), so
                gather per 512-edge chunk."""
                xsels = []
                for sub in range(chunks_per_blk):
                    xsel = qpool.tile([128, CHUNK], f16, tag="xsel")
                    nc.gpsimd.dma_gather(
                        xsel[:, :].rearrange("p (o i) -> p o i", o=1),
                        xpad_d[:, :],
                        jrep[b][:, sub * rows_per_chunk:(sub + 1) * rows_per_chunk],
                        CHUNK, nidx_reg, 128,
                        transpose=True,
                    )
                    xsels.append(xsel)
                return xsels

            def mlp_block(b, xsels):
                """Edge MLP for one block (lag-2 behind its screen)."""
                for sub in range(chunks_per_blk):
                    r0 = b * BLK + sub * rows_per_chunk
                    xsel = xsels[sub]
                    qps = ppQ.tile([c, CHUNK], f32, tag="qps")
                    nc.tensor.matmul(qps[:, :], w1bp[:, :], xsel[:, :],
                                     start=True, stop=False)
                    # += p_i via static one-hot: row sub*32+t selects cols
                    # [16t,16t+16) -> adds pTT[r0+t, :] to those edges
                    nc.tensor.matmul(
                        qps[:, :],
                        pTT[sub * 32:(sub + 1) * 32, b * c:(b + 1) * c],
                        onehot4[sub * 32:(sub + 1) * 32, :],
                        start=False, stop=True,
                        tile_position=(sub * 32, 0),
                    )
                    h1 = dpool.tile([128, CHUNK], f16, tag="h1")
                    nc.scalar.activation(
                        h1[:, :], qps[:, :], mybir.ActivationFunctionType.Relu,
                    )
                    ps2 = ppB.tile([128, CHUNK], f32, tag="mm2")
                    nc.tensor.matmul(ps2[:, :], w2[:, :], h1[:, :])
                    h2 = dpool.tile([128, CHUNK], f32, tag="h2")
                    nc.scalar.activation(
                        h2[:, :], ps2[:, :], mybir.ActivationFunctionType.Relu,
                        bias=b2s[:, :], scale=1.0 / k,
                    )
                    nc.vector.tensor_reduce(
                        out=outT[:, r0:r0 + rows_per_chunk],
                        in_=h2[:, :].rearrange("p (r k) -> p r k", k=k),
                        op=mybir.AluOpType.add,
                        axis=mybir.AxisListType.X,
                    )

            pend = {}
            for b in range(nblk + 2):
                if b < nblk:
                    cP = screen_block(b)
                    pend[b] = [cP, None]
                if b - 1 >= 0 and b - 1 < nblk:
                    j16f = extract_block(b - 1, pend[b - 1][0])
                    transpose_block(b - 1, j16f)
                    pend[b - 1][1] = mlp_gather(b - 1)
                if b - 2 >= 0:
                    mlp_block(b - 2, pend[b - 2][1])
                    del pend[b - 2]

            nc.sync.dma_start(out_d[:, :], outT[:, :])

    nc.finalize()
    return nc


def host_prep(x, W1, b1, W2, b2, n=N, d=D, c=C, k=K, rpc=RPC, ncores=NCORES):
    x = np.ascontiguousarray(np.asarray(x, dtype=np.float32))
    W1 = np.asarray(W1, dtype=np.float32)
    b1 = np.asarray(b1, dtype=np.float32)
    W2 = np.asarray(W2, dtype=np.float32)
    b2 = np.asarray(b2, dtype=np.float32)

    sq = np.sum(x * x, axis=1, dtype=np.float32)
    nwin = n // WIN
    ncoarse = nwin * 8
    a = np.float32(np.sqrt(2.0 * CSCALE))

    # moving operand: rows 0:d = a*x^T; d,d+1 = (sq-SHIFT) hi/lo; d+2.. = magic
    s = (sq - np.float32(SHIFT)).astype(np.float32)
    sh = s.astype(np.float16)
    sl = (s.astype(np.float64) - sh.astype(np.float64)).astype(np.float16)
    xaug = np.zeros((KA, n), dtype=np.float16)
    xaug[:d] = (x.T * a).astype(np.float16)
    xaug[d] = sh
    xaug[d + 1] = sl
    xaug[d + 2:d + 6] = np.float16(MAGR)

    unmw = np.full((4, 128), -MAGW, dtype=np.float16)
    unmr = np.full((4, WIN), MAGR, dtype=np.float16)
    iow = np.ones((1, 128), dtype=np.float16)
    ior = np.arange(WIN, dtype=np.float16).reshape(1, WIN)
    eye = np.eye(128, dtype=np.float32)
    iot = np.arange(WIN, dtype=np.float32)[None, :].repeat(128, axis=0)
    iot = np.ascontiguousarray(iot)

    # padded x rows for the HBM neighbor gather; col d holds the constant 1
    # that folds b1 into the per-chunk q matmul
    xpad = np.zeros((n, 128), dtype=np.float16)
    xpad[:, :d] = x.astype(np.float16)
    xpad[:, d] = np.float16(1.0)
    w1bp = np.zeros((128, c), dtype=np.float16)
    w1bp[:d] = W1[d:].astype(np.float16)
    w1bp[d] = b1.astype(np.float16)

    w1d = ((W1[:d] - W1[d:]) / a).astype(np.float16)
    oneh = np.zeros((128, CHUNK), dtype=np.float16)
    for p_ in range(128):
        t_ = p_ % 32
        oneh[p_, 16 * t_:16 * t_ + 16] = 1.0
    w2 = W2.astype(np.float16)
    b2s = (b2 / k).reshape(c, 1).astype(np.float32)

    in_maps = []
    for cid in range(ncores):
        rows = x[cid * rpc:(cid + 1) * rpc]
        wloc = np.zeros((KA, rpc), dtype=np.float16)
        wloc[:d] = (rows.T * a).astype(np.float16)
        wloc[d] = np.float16(-CSCALE)
        wloc[d + 1] = np.float16(-CSCALE)
        wloc[d + 2:d + 6] = np.float16(MAGW)
        in_maps.append(
            dict(
                xaug=xaug, wloc=np.ascontiguousarray(wloc), unmw=unmw,
                unmr=unmr, iow=iow, ior=ior, eye=eye,
                iot=iot,
                xpad=xpad, w1bp=w1bp, w1d=w1d, w2=w2, b2s=b2s,
                oneh=oneh,
            )
        )
    return in_maps


_NC_CACHE = {}


def kernel(x, W1, b1, W2, b2):
    _ensure_path()
    from concourse.bass_utils import run_bass_kernel_spmd

    key = "full"
    if key not in _NC_CACHE:
        _NC_CACHE[key] = build_program()
    nc = _NC_CACHE[key]

    in_maps = host_prep(x, W1, b1, W2, b2)
    res = run_bass_kernel_spmd(
        nc, in_maps, core_ids=list(range(NCORES)),
        trace=bool(int(os.environ.get("DGCNN_TRACE", "0"))),
    )
    out = np.empty((N, C), dtype=np.float32)
    for cid in range(NCORES):
        out[cid * RPC:(cid + 1) * RPC] = res.results[cid]["outT"].T
    if getattr(res, "exec_time_ns", None):
        kernel.last_exec_time_ns = res.exec_time_ns
    return out


kernel.last_exec_time_ns = None
